# revision 15
# baseline (speedup 1.0000x reference)
"""Trainium2 Bass kernel for nn_BlocksCore (RIMs BlocksCore fwd step).

Contract: kernel(**inputs) takes FULL unsharded inputs (np arrays, keyed as in
setup_inputs) and returns the FULL output tuple (hx_out [8192,1024] f32,
mask_full [8192,1024] f32), matching reference().

Strategy: pure data-parallel over batch (1024 samples/core on 8 cores).
Device layout is feature-major ([features, batch]); the host pre-transposes
inputs / post-transposes outputs and pre-fuses weights (Wv1[1] @ gru_wi).

v2: quad-packed PSUM layouts (scores/softmax for 4 query blocks share one
PSUM bank via col-groups; expansions ride 4-way row-group waves), longer
matmul accumulation chains, GPSIMD offload of SBUF-only elementwise work,
and full cross-tile DMA prefetch.
"""

import numpy as np
import ml_dtypes
from contextlib import ExitStack

import concourse.bass as bass
import concourse.bacc as bacc
import concourse.tile as tile
import concourse.mybir as mybir
from concourse.bass_utils import run_bass_kernel_spmd

AF = mybir.ActivationFunctionType
OP = mybir.AluOpType
f32 = mybir.dt.float32
bf16 = mybir.dt.bfloat16
BF = ml_dtypes.bfloat16

B, NINP, NHID = 8192, 256, 1024
NCORES = 8
BC = B // NCORES          # 1024 per core
F = 512                   # batch-tile columns
NT = BC // F              # 2 tiles
NB = 8                    # output blocks
BS = 128                  # block size


def _build_consts():
    """Constant 0/1 selector matrices."""
    c = {}
    # s1 quad-sum: prod[p] [128=(a2,e64), F] -> s1 rows 32b+j (j<8, 4 bases)
    m = np.zeros((4, 128, 104), np.float32)
    for p in range(4):
        for b in range(4):
            m[p, 0:64, 32 * b + 2 * p] = 1
            m[p, 64:128, 32 * b + 2 * p + 1] = 1
    c["c_s1q"] = m.transpose(1, 0, 2).reshape(128, 4 * 104)  # [:, p*104:(p+1)*104]

    # mask diff: diff[8i+j] = s1[j] - s1[i]
    pq = np.zeros((8, 64), np.float32)
    for i in range(8):
        for j in range(8):
            pq[j, 8 * i + j] += 1
            pq[i, 8 * i + j] -= 1
    c["pq"] = pq

    # rank quad: rank rows 32b+i = sum_j g[8i+j]
    r = np.zeros((64, 104), BF)
    for i in range(8):
        for j in range(8):
            for b in range(4):
                r[8 * i + j, 32 * b + i] = 1
    c["r64q"] = r

    # replication quad [8 rows at base 32b -> 128]: slice (k,b) = rows
    # 32b:32b+8, cols 128k:128k+128; selects row k -> all 128 out rows
    rq = np.zeros((104, 1024), BF)
    for k in range(8):
        for b in range(4):
            rq[32 * b + k, 128 * k:128 * (k + 1)] = 1
    c["repsq"] = rq

    # comm-attn QK sum: prod2(i,rr) rows (a,h,d) -> s row 4j+h, j=2rr+a
    m = np.zeros((4, 128, 32), BF)
    for rr in range(4):
        for a in range(2):
            for h in range(4):
                for d in range(16):
                    m[rr, 64 * a + 16 * h + d, 4 * (2 * rr + a) + h] = 1
    c["c_qksum"] = m.transpose(1, 0, 2).reshape(128, 128)  # [:, rr*32:(rr+1)*32]

    # den quad: expS_quad row 32b+4j+h -> den row 4b+h (sum over j)
    dq = np.zeros((128, 16), BF)
    for b in range(4):
        for j in range(8):
            for h in range(4):
                dq[32 * b + 4 * j + h, 4 * b + h] = 1
    c["c_denq"] = dq

    # erep quad: exp row 32b + 4(2rr+a)+h -> rep row 64a+16h+d
    eq = np.zeros((128, 512), BF)
    for b in range(4):
        for rr in range(4):
            for a in range(2):
                for h in range(4):
                    for d in range(16):
                        eq[32 * b + 4 * (2 * rr + a) + h,
                           128 * rr + 64 * a + 16 * h + d] = 1
    c["c_erepq"] = eq

    # rrep: recip row 4(i%4)+h -> rep row 64a+16h+d, i=2cc+a (same both quads)
    rr_ = np.zeros((16, 256), BF)
    for cc in range(4):
        for a in range(2):
            i = 2 * cc + a
            for h in range(4):
                for d in range(16):
                    rr_[4 * (i % 4) + h,
                        128 * (cc % 2) + 64 * a + 16 * h + d] = 1
    c["c_rrepq"] = rr_

    # fold: avp rows 64a+16h+d -> o rows 16h+d (sum over a)
    fold = np.zeros((128, 64), BF)
    for a in range(2):
        for h in range(4):
            for d in range(16):
                fold[64 * a + 16 * h + d, 16 * h + d] = 1
    c["fold"] = fold
    return c


_CONSTS = _build_consts()
_PROGRAM = None


def _build_program():
    nc = bacc.Bacc("TRN2", target_bir_lowering=False, debug=False)

    def din(name, shape, dt=bf16):
        return nc.dram_tensor(name, shape, dt, kind="ExternalInput")

    # per-core activations
    inpT = din("inpT", [NINP, BC])            # bf16
    inpTf = din("inpTf", [NINP, BC], f32)
    hxT = din("hxT", [NHID, BC], f32)
    hxTb = din("hxTb", [NHID, BC])            # bf16
    # weights (shared)
    wq1 = din("wq1", [128, 512], f32); wk1 = din("wk1", [128, 128], f32)
    wfu = din("wfu", [128, 6144]); wh = din("wh", [128, 3072])
    wq2 = din("wq2", [128, 512]); wk2 = din("wk2", [128, 512]); wv2 = din("wv2", [128, 512])
    fcg = din("fcg", [64, 256])
    b_rz = din("b_rz", [128, 16], f32)        # cols 2k: r, 2k+1: -z (negated)
    b_nbh = din("b_nbh", [128, 8], f32)
    b_nbi = din("b_nbi", [128, 8], f32)
    b_fg = din("b_fg", [128, 2], f32)
    # consts (mixed dtype)
    cdt = {"c_s1q": f32, "pq": f32}
    cs = {k: din("c_" + k, list(v.shape), cdt.get(k, bf16))
          for k, v in _CONSTS.items()}

    houtT = nc.dram_tensor("houtT", [NHID, BC], bf16, kind="ExternalOutput")
    mask8 = nc.dram_tensor("mask8", [8, BC], f32, kind="ExternalOutput")
    import os
    DEBUG = bool(os.environ.get("KDEBUG"))
    dbg = {}
    if DEBUG:
        dbg["hpr"] = nc.dram_tensor("d_hpr", [NHID, BC], f32, kind="ExternalOutput")
        dbg["zes"] = nc.dram_tensor("d_zes", [NHID, BC], f32, kind="ExternalOutput")
        dbg["exp"] = nc.dram_tensor("d_exp", [256, BC], f32, kind="ExternalOutput")
        dbg["oS"] = nc.dram_tensor("d_oS", [512, BC], f32, kind="ExternalOutput")
        dbg["recip"] = nc.dram_tensor("d_recip", [48, BC], f32, kind="ExternalOutput")
        dbg["rrep"] = nc.dram_tensor("d_rrep", [512, BC], f32, kind="ExternalOutput")
        dbg["on"] = nc.dram_tensor("d_on", [512, BC], f32, kind="ExternalOutput")

    with ExitStack() as ctx:
        tc = ctx.enter_context(tile.TileContext(nc))
        wp = ctx.enter_context(tc.tile_pool(name="wp", bufs=1))       # weights
        sb = ctx.enter_context(tc.tile_pool(name="sb", bufs=1))       # inputs (per-tile tags)
        tr = ctx.enter_context(tc.tile_pool(name="tr", bufs=1))       # transients bufs=1
        t2 = ctx.enter_context(tc.tile_pool(name="t2", bufs=2))       # transients bufs=2
        ak = ctx.enter_context(tc.tile_pool(name="ak", bufs=2))       # cycling transients
        a4 = ctx.enter_context(tc.tile_pool(name="a4", bufs=4))       # avp wave
        pw = ctx.enter_context(tc.tile_pool(name="pw", bufs=5, space="PSUM"))
        po = ctx.enter_context(tc.tile_pool(name="po", bufs=2, space="PSUM"))
        pm = ctx.enter_context(tc.tile_pool(name="pm", bufs=1, space="PSUM"))

        def wtile(dram, shape, dt=bf16, nchunk=1):
            t = wp.tile(shape, dt, tag=dram.name, name="t")
            if nchunk == 1:
                nc.sync.dma_start(t[:], dram.ap())
            else:
                cw = shape[1] // nchunk
                for i in range(nchunk):
                    nc.sync.dma_start(t[:, i * cw:(i + 1) * cw],
                                      dram.ap()[:, i * cw:(i + 1) * cw])
            return t

        # ---- phase-A-critical weights first ----
        W = {}
        W["wq1"] = wtile(wq1, [128, 512], f32, nchunk=2)
        W["wk1"] = wtile(wk1, [128, 128], f32)
        C = {}
        C["c_s1q"] = wtile(cs["c_s1q"], list(_CONSTS["c_s1q"].shape), f32)
        C["pq"] = wtile(cs["pq"], list(_CONSTS["pq"].shape), f32)
        C["r64q"] = wtile(cs["r64q"], list(_CONSTS["r64q"].shape))
        C["repsq"] = wtile(cs["repsq"], list(_CONSTS["repsq"].shape))

        # ---- inputs: tile-0 phase-A first, then GRU weights, then the rest ----
        inp_t = [[None, None] for _ in range(NT)]
        inpf_t = [[None, None] for _ in range(NT)]
        hx_t = [[None] * 8 for _ in range(NT)]
        hxb_t = [[None] * 8 for _ in range(NT)]

        def load_phaseA(t):
            sl = bass.ts(t, F)
            for cch in range(2):
                inpf_t[t][cch] = sb.tile([128, F], f32, tag=f"inpf{cch}_{t}",
                                         name=f"inpf{cch}_{t}")
                nc.sync.dma_start(inpf_t[t][cch][:], inpTf.ap()[bass.ts(cch, 128), sl])
            for k in range(8):
                hx_t[t][k] = sb.tile([128, F], f32, tag=f"hx{k}_{t}", name=f"hx{k}_{t}")
                nc.sync.dma_start(hx_t[t][k][:], hxT.ap()[bass.ts(k, 128), sl])

        def load_phaseBC(t):
            sl = bass.ts(t, F)
            for cch in range(2):
                inp_t[t][cch] = sb.tile([128, F], bf16, tag=f"inp{cch}_{t}",
                                        name=f"inp{cch}_{t}")
                nc.sync.dma_start(inp_t[t][cch][:], inpT.ap()[bass.ts(cch, 128), sl])
            for k in range(8):
                hxb_t[t][k] = sb.tile([128, F], bf16, tag=f"hxb{k}_{t}",
                                      name=f"hxb{k}_{t}")
                nc.sync.dma_start(hxb_t[t][k][:], hxTb.ap()[bass.ts(k, 128), sl])

        load_phaseA(0)
        W["wfu"] = wtile(wfu, [128, 6144], nchunk=4)
        W["wh"] = wtile(wh, [128, 3072], nchunk=2)
        load_phaseBC(0)

        # comm-attn weights + consts
        for d, sh in [(wq2, [128, 512]), (wk2, [128, 512]), (wv2, [128, 512])]:
            W[d.name] = wtile(d, sh)
        fcg_t = wp.tile([128, 256], bf16, tag="fcg", name="fcg")
        nc.sync.dma_start(fcg_t[0:64, :], fcg.ap())
        nc.sync.dma_start(fcg_t[64:128, :], fcg.ap())
        W["fcg"] = fcg_t
        for d, sh in [(b_rz, [128, 16]), (b_nbh, [128, 8]), (b_nbi, [128, 8]),
                      (b_fg, [128, 2])]:
            W[d.name] = wtile(d, sh, f32)
        for k in ("c_qksum", "c_denq", "c_erepq", "c_rrepq", "fold"):
            C[k] = wtile(cs[k], list(_CONSTS[k].shape))
        load_phaseA(1)
        load_phaseBC(1)

        # ================= compute =================
        for t in range(NT):
            sl = bass.ts(t, F)
            inpf, inpb = inpf_t[t], inp_t[t]
            hx, hxb = hx_t[t], hxb_t[t]

            # ---- phase A: input attention scores + mask ----
            # kkRep [128,F]: rows 0:64 and 64:128 both = inp @ Wk1[1]
            kk_ps = pw.tile([128, F], f32, tag="pw", name="pw")
            for cch in range(2):
                nc.tensor.matmul(kk_ps[0:64, :], W["wk1"][:, bass.ts(cch, 64)],
                                 inpf[cch][:], start=(cch == 0), stop=(cch == 1))
            for cch in range(2):
                nc.tensor.matmul(kk_ps[64:128, :], W["wk1"][:, bass.ts(cch, 64)],
                                 inpf[cch][:], start=(cch == 0), stop=(cch == 1),
                                 tile_position=(0, 64))
            kkS = tr.tile([128, F], f32, tag="kkS", name="kkS")
            nc.scalar.copy(kkS[:], kk_ps[:])

            s1_ps = pm.tile([104, F], f32, tag="pm", name="pm")
            for p in range(4):
                q_ps = pw.tile([128, F], f32, tag="pw", name="pw")
                nc.tensor.matmul(q_ps[0:64, :], W["wq1"][:, bass.ts(2 * p, 64)],
                                 hx[2 * p][:], start=True, stop=True)
                nc.tensor.matmul(q_ps[64:128, :], W["wq1"][:, bass.ts(2 * p + 1, 64)],
                                 hx[2 * p + 1][:], start=True, stop=True,
                                 tile_position=(0, 64))
                pr = ak.tile([128, F], f32, tag="prod", name="prod")
                nc.vector.tensor_tensor(pr[:], q_ps[:], kkS[:], OP.mult)
                nc.tensor.matmul(s1_ps[:], C["c_s1q"][:, p * 104:(p + 1) * 104],
                                 pr[:], start=(p == 0), stop=(p == 3))

            # sigmoid(s1/8) quad-replicated (feeds att scaling waves)
            s1sig = tr.tile([104, F], bf16, tag="s1sig", name="s1sig")
            nc.scalar.activation(s1sig[:], s1_ps[:], AF.Sigmoid, scale=0.125)
            # f32 s1 (rows 0:8) for exact mask ranking
            s1S = tr.tile([8, F], f32, tag="s1S", name="s1S")
            nc.scalar.copy(s1S[:], s1_ps[0:8, :])

            diff_ps = pw.tile([64, F], f32, tag="pw", name="pw")
            nc.tensor.matmul(diff_ps[:], C["pq"][:], s1S[:], start=True, stop=True)
            g = tr.tile([64, F], bf16, tag="g", name="g")
            nc.vector.tensor_single_scalar(g[:], diff_ps[:], 0.0, OP.is_gt)
            rank_ps = pm.tile([104, F], f32, tag="pm", name="pm")
            nc.tensor.matmul(rank_ps[:], C["r64q"][:], g[:], start=True, stop=True)
            m8 = tr.tile([104, F], bf16, tag="m8", name="m8")
            nc.vector.tensor_single_scalar(m8[:], rank_ps[:], 3.5, OP.is_le)
            nc.gpsimd.dma_start(mask8.ap()[:, sl], m8[0:8, :])

            # att1 replication waves (4-way row groups) + mask replication
            attS = [None] * 8
            for k in range(8):
                b = k % 4
                a_ps = pw.tile([128, F], f32, tag="pw", name="pw")
                nc.tensor.matmul(a_ps[:],
                                 C["repsq"][32 * b:32 * b + 8, bass.ts(k, 128)],
                                 s1sig[32 * b:32 * b + 8, :], start=True, stop=True,
                                 tile_position=(32 * b, 0))
                attS[k] = tr.tile([128, F], bf16, tag=f"attS{k}", name=f"attS{k}")
                nc.scalar.copy(attS[k][:], a_ps[:])
            mrepS = [None] * 8
            for k in range(8):
                b = k % 4
                mr_ps = pw.tile([128, F], f32, tag="pw", name="pw")
                nc.tensor.matmul(mr_ps[:],
                                 C["repsq"][32 * b:32 * b + 8, bass.ts(k, 128)],
                                 m8[32 * b:32 * b + 8, :], start=True, stop=True,
                                 tile_position=(32 * b, 0))
                mrepS[k] = t2.tile([128, F], bf16, tag=f"mrepS{k}", name=f"mrepS{k}")
                nc.scalar.copy(mrepS[k][:], mr_ps[:])

            # ---- phase B: block GRU ----
            hpr = [None] * 8   # h' bf16
            zes = [None] * 8   # z'*(n-h) bf16
            for k in range(8):
                xk = [None, None]
                for cch in range(2):
                    xk[cch] = ak.tile([128, F], bf16, tag=f"xk{cch}", name=f"xk{cch}")
                    nc.vector.tensor_tensor(xk[cch][:], attS[k][:], inpb[cch][:],
                                            OP.mult)
                kb = k * 384
                gate_ps = {}
                for gi, gn in enumerate(("r", "z", "n")):
                    gp = pw.tile([128, F], f32, tag="pw", name="pw")
                    last_wfu = gn == "n"
                    for cch in range(2):
                        nc.tensor.matmul(gp[:], W["wfu"][:, cch * 3072 + kb + gi * 128:
                                                         cch * 3072 + kb + gi * 128 + 128],
                                         xk[cch][:], start=(cch == 0),
                                         stop=(last_wfu and cch == 1))
                    if not last_wfu:
                        nc.tensor.matmul(gp[:], W["wh"][:, kb + gi * 128: kb + gi * 128 + 128],
                                         hxb[k][:], start=False, stop=True)
                    gate_ps[gn] = gp
                hn_ps = pw.tile([128, F], f32, tag="pw", name="pw")
                nc.tensor.matmul(hn_ps[:], W["wh"][:, kb + 256: kb + 384],
                                 hxb[k][:], start=True, stop=True)

                r = ak.tile([128, F], bf16, tag="r", name="r")
                nc.scalar.activation(r[:], gate_ps["r"][:], AF.Sigmoid,
                                     bias=W["b_rz"][:, 2 * k: 2 * k + 1])
                zp = ak.tile([128, F], bf16, tag="zp", name="zp")
                nc.scalar.activation(zp[:], gate_ps["z"][:], AF.Sigmoid, scale=-1.0,
                                     bias=W["b_rz"][:, 2 * k + 1: 2 * k + 2])
                rhn = ak.tile([128, F], bf16, tag="rhn", name="rhn")
                nc.vector.scalar_tensor_tensor(rhn[:], hn_ps[:],
                                               W["b_nbh"][:, k: k + 1], r[:],
                                               OP.add, OP.mult)
                npre = ak.tile([128, F], bf16, tag="npre", name="npre")
                nc.vector.tensor_tensor(npre[:], rhn[:], gate_ps["n"][:], OP.add)
                n = ak.tile([128, F], bf16, tag="n", name="n")
                nc.scalar.activation(n[:], npre[:], AF.Tanh,
                                     bias=W["b_nbi"][:, k: k + 1])
                e = ak.tile([128, F], bf16, tag="e", name="e")
                nc.vector.tensor_tensor(e[:], n[:], hxb[k][:], OP.subtract)
                zes[k] = t2.tile([128, F], bf16, tag=f"ze{k}", name=f"ze{k}")
                nc.vector.tensor_tensor(zes[k][:], zp[:], e[:], OP.mult)
                hpr[k] = tr.tile([128, F], bf16, tag=f"hpr{k}", name=f"hpr{k}")
                nc.vector.tensor_tensor(hpr[k][:], hxb[k][:], zes[k][:], OP.add)
                if DEBUG:
                    nc.gpsimd.dma_start(dbg["hpr"].ap()[bass.ts(k, 128), sl], hpr[k][:])
                    nc.gpsimd.dma_start(dbg["zes"].ap()[bass.ts(k, 128), sl], zes[k][:])

            # ---- phase C: communication attention ----
            k2S = [None] * 4
            v2S = [None] * 4
            for rr in range(4):
                kp = pw.tile([128, F], f32, tag="pw", name="pw")
                nc.tensor.matmul(kp[0:64, :], W["wk2"][:, bass.ts(2 * rr, 64)],
                                 hpr[2 * rr][:], start=True, stop=True)
                nc.tensor.matmul(kp[64:128, :], W["wk2"][:, bass.ts(2 * rr + 1, 64)],
                                 hpr[2 * rr + 1][:], start=True, stop=True,
                                 tile_position=(0, 64))
                k2S[rr] = tr.tile([128, F], bf16, tag=f"k2S{rr}", name=f"k2S{rr}")
                nc.scalar.copy(k2S[rr][:], kp[:])
                vp = pw.tile([128, F], f32, tag="pw", name="pw")
                nc.tensor.matmul(vp[0:64, :], W["wv2"][:, bass.ts(2 * rr, 64)],
                                 hpr[2 * rr][:], start=True, stop=True)
                nc.tensor.matmul(vp[64:128, :], W["wv2"][:, bass.ts(2 * rr + 1, 64)],
                                 hpr[2 * rr + 1][:], start=True, stop=True,
                                 tile_position=(0, 64))
                v2S[rr] = tr.tile([128, F], bf16, tag=f"v2S{rr}", name=f"v2S{rr}")
                nc.scalar.copy(v2S[rr][:], vp[:])

            # scores + softmax numerators, quad-packed: i%4 -> col/row group
            expQ = [None, None]
            for q in range(2):
                s_quad = pm.tile([128, F], f32, tag="pm", name="pm")
                for bq in range(4):
                    i = 4 * q + bq
                    qp = pw.tile([128, F], f32, tag="pw", name="pw")
                    nc.tensor.matmul(qp[0:64, :], W["wq2"][:, bass.ts(i, 64)],
                                     hpr[i][:], start=True, stop=True)
                    nc.tensor.matmul(qp[64:128, :], W["wq2"][:, bass.ts(i, 64)],
                                     hpr[i][:], start=True, stop=True,
                                     tile_position=(0, 64))
                    qdS = ak.tile([128, F], bf16, tag="qdS", name="qdS")
                    nc.scalar.copy(qdS[:], qp[:])
                    for rr in range(4):
                        pr2 = ak.tile([128, F], bf16, tag="prod2", name="prod2")
                        nc.vector.tensor_tensor(pr2[:], qdS[:], k2S[rr][:], OP.mult)
                        nc.tensor.matmul(s_quad[32 * bq:32 * bq + 32, :],
                                         C["c_qksum"][:, bass.ts(rr, 32)], pr2[:],
                                         start=(rr == 0),
                                         stop=(rr == 3),
                                         tile_position=(0, 32 * bq),
                                         skip_group_check=True)
                expQ[q] = tr.tile([128, F], bf16, tag=f"expQ{q}", name=f"expQ{q}")
                nc.scalar.activation(expQ[q][:], s_quad[:], AF.Exp, scale=0.25)
                if DEBUG:
                    nc.gpsimd.dma_start(dbg["exp"].ap()[bass.ts(q, 128), sl], expQ[q][:])

            # softmax denominators, one small PSUM tile per quad (base 0)
            recipQ = [None, None]
            for q in range(2):
                den_ps = pm.tile([16, F], f32, tag="pm", name="pm")
                nc.tensor.matmul(den_ps[:], C["c_denq"][:], expQ[q][:],
                                 start=True, stop=True)
                recipF = ak.tile([16, F], f32, tag="recipF", name="recipF")
                with nc.allow_low_precision(reason="softmax denom ~8, approx ok"):
                    nc.vector.reciprocal_approx_fast(recipF[:], den_ps[:])
                recipQ[q] = ak.tile([16, F], bf16, tag="recipSq", name="recipSq")
                nc.scalar.copy(recipQ[q][:], recipF[:])
                if DEBUG:
                    nc.sync.dma_start(dbg["recip"].ap()[32 * q:32 * q + 16, sl],
                                      recipF[:])

            # attention-weighted values, 4-way erep waves + fold accumulation
            oS = [None] * 4
            for q in range(2):
                on_ps = [po.tile([128, F], f32, tag="po", name="po") for _ in range(2)]
                for rr in range(4):
                    er_ps = [None] * 4
                    for bq in range(4):
                        er_ps[bq] = pw.tile([128, F], f32, tag="pw", name="pw")
                        nc.tensor.matmul(
                            er_ps[bq][:],
                            C["c_erepq"][32 * bq:32 * bq + 32, bass.ts(rr, 128)],
                            expQ[q][32 * bq:32 * bq + 32, :],
                            start=True, stop=True, tile_position=(32 * bq, 0))
                    avps = [None] * 4
                    for bq in range(4):
                        avps[bq] = a4.tile([128, F], bf16, tag="avp", name="avp")
                        nc.vector.tensor_tensor(avps[bq][:], er_ps[bq][:],
                                                v2S[rr][:], OP.mult)
                    for bq in range(4):
                        i = 4 * q + bq
                        cc, a = divmod(i, 2)
                        nc.tensor.matmul(on_ps[cc % 2][64 * a:64 * a + 64, :],
                                         C["fold"][:], avps[bq][:],
                                         start=(rr == 0),
                                         stop=(rr == 3),
                                         tile_position=(0, 64 * a),
                                         skip_group_check=True)
                for ccl in range(2):
                    cc = 2 * q + ccl
                    rp = pw.tile([128, F], f32, tag="pw", name="pw")
                    nc.tensor.matmul(
                        rp[:],
                        C["c_rrepq"][:, 128 * ccl:128 * ccl + 128],
                        recipQ[q][:],
                        start=True, stop=True)
                    rrepS = ak.tile([128, F], bf16, tag="rrepS", name="rrepS")
                    nc.scalar.copy(rrepS[:], rp[:])
                    if DEBUG:
                        onD = tr.tile([128, F], f32, tag="onD", name="onD")
                        nc.scalar.copy(onD[:], on_ps[ccl][:])
                        nc.sync.dma_start(dbg["on"].ap()[bass.ts(cc, 128), sl], onD[:])
                        nc.gpsimd.dma_start(dbg["rrep"].ap()[bass.ts(cc, 128), sl], rrepS[:])
                    oS[cc] = tr.tile([128, F], bf16, tag=f"oS{ccl}", name=f"oS{ccl}")
                    nc.vector.tensor_tensor(oS[cc][:], on_ps[ccl][:], rrepS[:],
                                            OP.mult)
                    if DEBUG:
                        nc.gpsimd.dma_start(dbg["oS"].ap()[bass.ts(cc, 128), sl], oS[cc][:])

                # fc / gate for the 4 blocks of this quad (paired row groups)
                for ccl in range(2):
                    cc = 2 * q + ccl
                    fc_ps = [None, None]
                    gt_ps = [None, None]
                    for a in range(2):
                        osrc = oS[cc][64 * a:64 * a + 64, :]
                        wsl = W["fcg"][64 * a:64 * a + 64, :]
                        fc_ps[a] = pw.tile([128, F], f32, tag="pw", name="pw")
                        nc.tensor.matmul(fc_ps[a][:], wsl[:, 0:128], osrc,
                                         start=True, stop=True,
                                         tile_position=(64 * a, 0))
                    for a in range(2):
                        osrc = oS[cc][64 * a:64 * a + 64, :]
                        wsl = W["fcg"][64 * a:64 * a + 64, :]
                        gt_ps[a] = pw.tile([128, F], f32, tag="pw", name="pw")
                        nc.tensor.matmul(gt_ps[a][:], wsl[:, 128:256], osrc,
                                         start=True, stop=True,
                                         tile_position=(64 * a, 0))
                    for a in range(2):
                        k = 2 * cc + a
                        th = ak.tile([128, F], bf16, tag="th", name="th")
                        nc.scalar.activation(th[:], fc_ps[a][:], AF.Tanh,
                                             bias=W["b_fg"][:, 0:1])
                        sg = ak.tile([128, F], bf16, tag="sg", name="sg")
                        nc.scalar.activation(sg[:], gt_ps[a][:], AF.Sigmoid,
                                             bias=W["b_fg"][:, 1:2])
                        att = ak.tile([128, F], bf16, tag="att", name="att")
                        nc.vector.tensor_tensor(att[:], sg[:], th[:], OP.mult)

                        delta = ak.tile([128, F], bf16, tag="delta", name="delta")
                        nc.vector.tensor_tensor(delta[:], zes[k][:], att[:], OP.add)
                        mdelta = ak.tile([128, F], bf16, tag="mdelta", name="mdelta")
                        nc.vector.tensor_tensor(mdelta[:], mrepS[k][:], delta[:],
                                                OP.mult)
                        outk = ak.tile([128, F], bf16, tag="outk", name="outk")
                        nc.vector.tensor_tensor(outk[:], hx[k][:], mdelta[:], OP.add)
                        nc.sync.dma_start(houtT.ap()[bass.ts(k, 128), sl], outk[:])

    nc.compile()
    return nc


def _prep_shared(inputs):
    """Host-side weight prep (shared across cores)."""
    g = lambda k: np.asarray(inputs[k], np.float32)
    Wq1, Wk1, Wv1 = g("Wq1"), g("Wk1"), g("Wv1")
    Wq2, Wk2, Wv2 = g("Wq2"), g("Wk2"), g("Wv2")
    fc_w, fc_b, gate_w, gate_b = g("fc_w"), g("fc_b"), g("gate_w"), g("gate_b")
    gwi, gwh, gbi, gbh = g("gru_wi"), g("gru_wh"), g("gru_bi"), g("gru_bh")

    sh = {}
    sh["wq1"] = np.ascontiguousarray(Wq1.transpose(1, 0, 2).reshape(128, 512))
    sh["wk1"] = np.ascontiguousarray(
        Wk1[1].reshape(2, 128, 64).transpose(1, 0, 2).reshape(128, 128))
    wf = np.einsum("de,kef->kdf", Wv1[1], gwi)           # [8, 256, 384]
    sh["wfu"] = np.ascontiguousarray(
        wf.reshape(8, 2, 128, 384).transpose(2, 1, 0, 3).reshape(128, 6144)).astype(BF)
    sh["wh"] = np.ascontiguousarray(gwh.transpose(1, 0, 2).reshape(128, 3072)).astype(BF)
    sh["wq2"] = np.ascontiguousarray(Wq2.transpose(1, 0, 2).reshape(128, 512)).astype(BF)
    sh["wk2"] = np.ascontiguousarray(Wk2.transpose(1, 0, 2).reshape(128, 512)).astype(BF)
    sh["wv2"] = np.ascontiguousarray(Wv2.transpose(1, 0, 2).reshape(128, 512)).astype(BF)
    fg = np.zeros((64, 256), np.float32)
    fg[:, 0:128] = fc_w
    fg[:, 128:256] = gate_w
    sh["fcg"] = fg.astype(BF)

    brz = np.zeros((128, 16), np.float32)
    bnbh = np.zeros((128, 8), np.float32)
    bnbi = np.zeros((128, 8), np.float32)
    for k in range(8):
        brz[:, 2 * k] = gbi[k, 0:128] + gbh[k, 0:128]
        brz[:, 2 * k + 1] = -(gbi[k, 128:256] + gbh[k, 128:256])
        bnbh[:, k] = gbh[k, 256:384]
        bnbi[:, k] = gbi[k, 256:384]
    sh["b_rz"], sh["b_nbh"], sh["b_nbi"] = brz, bnbh, bnbi
    bfg = np.zeros((128, 2), np.float32)
    bfg[:, 0] = fc_b
    bfg[:, 1] = gate_b
    sh["b_fg"] = bfg
    for k, v in _CONSTS.items():
        sh["c_" + k] = v
    return sh


def kernel(**inputs):
    global _PROGRAM
    if _PROGRAM is None:
        _PROGRAM = _build_program()
    nc = _PROGRAM

    inp = np.asarray(inputs["inp"], np.float32)
    hx = np.asarray(inputs["hx"], np.float32)

    sh = _prep_shared(inputs)
    in_maps = []
    for c in range(NCORES):
        s = slice(c * BC, (c + 1) * BC)
        m = dict(sh)
        inpTc = np.ascontiguousarray(inp[s].T)
        m["inpT"] = inpTc.astype(BF)
        m["inpTf"] = inpTc
        hxTc = np.ascontiguousarray(hx[s].T)
        m["hxT"] = hxTc
        m["hxTb"] = hxTc.astype(BF)
        in_maps.append(m)

    res = run_bass_kernel_spmd(nc, in_maps, list(range(NCORES)))
    hx_out = np.empty((B, NHID), np.float32)
    mask_full = np.empty((B, NHID), np.float32)
    for c in range(NCORES):
        s = slice(c * BC, (c + 1) * BC)
        hx_out[s] = res.results[c]["houtT"].T.astype(np.float32)
        mask_full[s] = np.repeat(res.results[c]["mask8"].T, 128, axis=1)
    return hx_out, mask_full
